# revision 2
# baseline (speedup 1.0000x reference)
"""Trainium2 Bass kernel for BertInfiniSelfAttention — v2 design.

Sharding (8 cores): core c = (batch b = c//4, kv-quarter q = c%4).
Each core owns batch b and kv heads {3q, 3q+1, 3q+2} (KVL = 2304 of the
9216 concatenated kv positions), computes the full Q projection for its
batch, the K/V projections + memory matmuls for its 3 kv heads, then
flash-style partial attention for all 12 q heads against its local KV.
Host sums partial (numerator, denominator) over the 4 kv-quarters per
batch and divides.

Device-side schedule per core (single batch):
  A: Q proj kt-chunked (starts as soon as the first wq/hT DMA chunks
     land; 6 head-pair accumulators of [128, 384], two s-halves), then
     K/V proj for the 3 kv heads.
  B: kcT [128, 2304] with kv-duplicated halves via concurrent col-tiled
     matmul pairs.
  C: 6 sweeps (one per q-head pair).  Sweep p: 18 kv-tile slots; each
     slot does a CONCURRENT row-tiled scores pair (tile_position
     (0,0)/(64,0)) into a [128, 1536] psum tile, exp on the Scalar
     engine (ACT exp) or the Vector engine (custom DVE op
     exp(x) ~ (1+x/64)^64, 6 SQUARE stages) per a 10:8 split.  The vc
     memory matmuls fill sweep 0's PE slack; ctx chains are
     software-pipelined: A(p) groups 0-7 inline at slots 10-17 of sweep
     p, A(p) tail + B(p) during sweep p+1 slots 0-10, one [65, 768]
     psum accumulator (row 64 = em-weighted softmax denominator).

exp approximation error (end-to-end vs jax reference): ~7e-4 — scores
are small (|x| < 1.2) and the softmax near-uniform, so the squaring
bias largely cancels in num/den.
"""

import numpy as np

B, S, H, NH, D = 2, 768, 768, 12, 64
P = 128
NCORES = 8
NQUART = 4              # kv quarters
HK = 3                  # kv heads per core
KVL = HK * S            # 2304 local kv
NKT = KVL // P          # 18 kv tiles
NT = S // P             # 6 s/H tiles
DP1 = D + 1
NPAIR = NH // 2         # 6 q-head pairs
SH = S // 2             # 384, Q-proj s-half

# which kv tiles the Scalar engine exps (rest go to the Vector engine)
ACT_TILES = frozenset((0, 2, 4, 6, 8, 10, 12, 14, 16, 17))

_PROGRAM = None
TRACE = False
LAST_RESULTS = None


def _bank_pieces(lo, hi):
    """Split [lo,hi) free-dim range at 512-fp32 PSUM bank boundaries."""
    out = []
    while lo < hi:
        nxt = min(hi, (lo // 512 + 1) * 512)
        out.append((lo, nxt))
        lo = nxt
    return out


def _exp_ref(in0, in1, s0, s1, imm2):
    t = (in0 * s0 + 1.0).astype(np.float32)
    for _ in range(6):
        t = t * t
    return t


def _make_exp_op():
    from concourse import dve_ops as DO
    from concourse.dve_spec import Spec, Src0, C0, One, sq

    for o in DO.OPS:
        if o.name == "EXP_SQ64_ANT":
            return o
    body = sq(sq(sq(sq(sq(sq(Src0 * C0 + One))))))
    op = DO.DveOp(
        "EXP_SQ64_ANT",
        Spec(body=body, reference=_exp_ref),
        subdim=False,
        uops_sha={"v3": "52f44558ff295216", "v4": "63d0fb0e3de70366"},
    )
    DO.OPS.append(op)
    DO._SUB_OPCODE_FOR_NAME[op.name] = DO._CUSTOM_DVE_ROW_BASE + len(DO.OPS) - 1
    return op


def _build_program():
    from contextlib import ExitStack
    from itertools import zip_longest

    import concourse.bacc as bacc
    import concourse.mybir as mybir
    import concourse.tile as tile

    exp_op = _make_exp_op()

    F32 = mybir.dt.float32
    F16 = mybir.dt.float16
    EXP = mybir.ActivationFunctionType.Exp

    nc = bacc.Bacc("TRN2", target_bir_lowering=False, debug=False,
                   num_devices=NCORES)

    hT = nc.declare_dram_parameter("hT", [H, S], F16, isOutput=False)
    wq = nc.declare_dram_parameter("wq", [H, H], F16, isOutput=False)
    bq_d = nc.declare_dram_parameter("bq_d", [P, NPAIR], F32, isOutput=False)
    wkv = nc.declare_dram_parameter("wkv", [H, 6 * D], F16, isOutput=False)
    bkv_d = nc.declare_dram_parameter("bkv_d", [1, 6 * D], F16, isOutput=False)
    mkt = nc.declare_dram_parameter("mkt", [S, KVL], F16, isOutput=False)
    mvt = nc.declare_dram_parameter("mvt", [S, KVL], F16, isOutput=False)
    em_d = nc.declare_dram_parameter("em_d", [P, NKT], F32, isOutput=False)
    em16_d = nc.declare_dram_parameter("em16_d", [P, NKT], F16, isOutput=False)
    ones_d = nc.declare_dram_parameter("ones_d", [1, S], F16, isOutput=False)
    out_d = nc.declare_dram_parameter("out_d", [NH, DP1, S], F32, isOutput=True)

    with tile.TileContext(nc) as tc, ExitStack() as ctx:
        const = ctx.enter_context(tc.tile_pool(name="const", bufs=1))

        qT = const.tile([P, NPAIR * S], F16, name="qT")
        kcT = const.tile([P, KVL], F16, name="kcT")
        vca = const.tile([P, NKT * DP1], F16, name="vca")
        ones = const.tile([1, S], F16, name="ones")
        bq_s = const.tile([P, NPAIR], F32, name="bq_s")
        bkv_s = const.tile([1, 6 * D], F16, name="bkv_s")
        act_warm = const.tile([1, 8], F16, name="act_warm")

        nc.gpsimd.dma_start(ones[:], ones_d[:])
        nc.gpsimd.dma_start(bq_s[:], bq_d[:])
        nc.gpsimd.dma_start(bkv_s[:], bkv_d[:])
        # prefill the em (denominator) columns of vca straight from DRAM
        nc.gpsimd.dma_start(
            vca[:].rearrange("p (t c) -> p t c", c=DP1)[:, :, D:DP1],
            em16_d[:].rearrange("p (t c) -> p t c", c=1))
        # warm the exp table (ACT_TABLE_LOAD ~2.7us) during phase A
        nc.scalar.activation(act_warm[:], ones[:, 0:8], EXP, scale=0.125)

        # ---- long-lived inputs (kv3 + mvt live until vc is done) ----
        iov = ctx.enter_context(tc.tile_pool(name="iov", bufs=1))
        kv3 = iov.tile([P, NT * 6 * D], F16, name="kv3")
        mvt_s = iov.tile([P, NT * KVL], F16, name="mvt_s")

        with tc.tile_pool(name="iok", bufs=1) as iok:
            mkt_s = iok.tile([P, NT * KVL], F16, name="mkt_s")

            # ---- Phase A ----
            with tc.tile_pool(name="ioa", bufs=1) as ioa:
                wq_s = ioa.tile([P, NT * H], F16, name="wq_s")
                hT_s = ioa.tile([P, NT * S], F16, name="hT_s")
                wkv_s = ioa.tile([P, NT * 6 * D], F16, name="wkv_s")

                # DMA queues: sync = wq, wkv then outputs; scalar = hT then
                # mvt; gpsimd = consts then mkt.  wq/hT are issued as
                # per-chunk DMAs so the kt-chunked Q proj can start as soon
                # as chunk kt lands.
                nc.sync.dma_start(
                    wq_s[:].rearrange("p (kt c) -> p kt c", c=H),
                    wq[:].rearrange("(kt p) c -> p kt c", p=P))
                nc.scalar.dma_start(
                    hT_s[:].rearrange("p (kt c) -> p kt c", c=S),
                    hT[:].rearrange("(kt p) c -> p kt c", p=P))
                nc.sync.dma_start(
                    wkv_s[:].rearrange("p (kt c) -> p kt c", c=6 * D),
                    wkv[:].rearrange("(kt p) c -> p kt c", p=P))
                for h in range(HK):
                    for st in range(NT):
                        nc.gpsimd.dma_start(
                            mkt_s[:, st * KVL + h * S: st * KVL + (h + 1) * S],
                            mkt[st * P:(st + 1) * P, h * S:(h + 1) * S])
                for h in range(HK):
                    for st in range(NT):
                        nc.scalar.dma_start(
                            mvt_s[:, st * KVL + h * S: st * KVL + (h + 1) * S],
                            mvt[st * P:(st + 1) * P, h * S:(h + 1) * S])

                # Q projection (pair-outer) + K/V projection
                with tc.tile_pool(name="aps", bufs=2, space="PSUM") as aps:
                    for t in range(NPAIR):
                        q_ps = aps.tile([P, S], F32, name="q_ps", tag="q_ps")
                        for lo, hi in _bank_pieces(0, S):
                            for kt in range(NT):
                                nc.tensor.matmul(
                                    q_ps[:, lo:hi],
                                    wq_s[:, kt * H + t * P: kt * H + (t + 1) * P],
                                    hT_s[:, kt * S + lo: kt * S + hi],
                                    start=(kt == 0), stop=(kt == NT - 1))
                        if t % 2 == 0:
                            nc.vector.tensor_scalar_add(
                                qT[:, t * S:(t + 1) * S], q_ps[:],
                                bq_s[:, t:t + 1])
                        else:
                            nc.scalar.activation(
                                qT[:, t * S:(t + 1) * S], q_ps[:],
                                mybir.ActivationFunctionType.Identity,
                                bias=bq_s[:, t:t + 1])

                    for st in range(NT):
                        kv_ps = aps.tile([P, 6 * D], F32, name="kv_ps",
                                         tag="kv_ps")
                        for kt in range(NT):
                            nc.tensor.matmul(
                                kv_ps[:],
                                hT_s[:, kt * S + st * P: kt * S + (st + 1) * P],
                                wkv_s[:, kt * 6 * D:(kt + 1) * 6 * D],
                                start=(kt == 0), stop=False)
                        nc.tensor.matmul(kv_ps[:], ones[:, 0:P], bkv_s[:],
                                         start=False, stop=True)
                        nc.vector.tensor_copy(
                            kv3[:, st * 6 * D:(st + 1) * 6 * D], kv_ps[:])

            # ---- Phase B: kc (kv-duplicated halves, concurrent col pairs)
            with tc.tile_pool(name="kcps", bufs=2, space="PSUM") as kcps:
                for h in range(HK):
                    kc_ps = kcps.tile([P, S], F32, name="kc_ps", tag="kc_ps")
                    for lo, hi in _bank_pieces(0, S):
                        for st in range(NT):
                            lhsT = kv3[:, st * 6 * D + h * D:
                                       st * 6 * D + (h + 1) * D]
                            rhs = mkt_s[:, st * KVL + h * S + lo:
                                        st * KVL + h * S + hi]
                            nc.tensor.matmul(
                                kc_ps[0:D, lo:hi], lhsT, rhs,
                                start=(st == 0), stop=(st == NT - 1))
                            nc.tensor.matmul(
                                kc_ps[D:P, lo:hi], lhsT, rhs,
                                start=(st == 0), stop=(st == NT - 1),
                                tile_position=(0, D))
                    nc.vector.tensor_copy(kcT[:, h * S:(h + 1) * S], kc_ps[:])

        # ---- Phase C ----
        prp = ctx.enter_context(tc.tile_pool(name="prp", bufs=38))
        scps = ctx.enter_context(tc.tile_pool(name="scps", bufs=2,
                                              space="PSUM"))
        stg = ctx.enter_context(tc.tile_pool(name="stg", bufs=4))

        def emit_scores(p, t):
            """Concurrent row-tiled pair: even head -> cols 0:768, odd
            head -> cols 768:1536 of a [128, 1536] psum tile."""
            sc = scps.tile([P, 2 * S], F32, name="sc", tag="sc")
            kc_lo = kcT[0:D, t * P:(t + 1) * P]
            kc_hi = kcT[D:P, t * P:(t + 1) * P]
            for pa, pb in zip_longest(_bank_pieces(0, S),
                                      _bank_pieces(S, 2 * S)):
                if pa is not None:
                    lo, hi = pa
                    nc.tensor.matmul(sc[:, lo:hi], kc_lo,
                                     qT[0:D, p * S + lo: p * S + hi],
                                     start=True, stop=True)
                if pb is not None:
                    lob, hib = pb
                    nc.tensor.matmul(
                        sc[:, lob:hib], kc_hi,
                        qT[D:P, p * S + lob - S: p * S + hib - S],
                        start=True, stop=True)
            pr = prp.tile([P, 2 * S], F16, name="pr", tag="pr")
            if t in ACT_TILES:
                nc.scalar.activation(pr[:], sc[:], EXP, scale=0.125)
            else:
                nc.vector._custom_dve(exp_op, out=pr[:], in0=sc[:],
                                      s0=0.125 / 64)
            return pr

        def emit_ctx(t, pr, half, ps, first, last):
            base = half * S
            for lo, hi in _bank_pieces(0, S):
                nc.tensor.matmul(
                    ps[:, lo:hi], vca[:, t * DP1:(t + 1) * DP1],
                    pr[:, base + lo: base + hi],
                    start=first, stop=last)

        def flush_ctx(ps, head):
            st_t = stg.tile([DP1, S], F32, name="st_t", tag="st")
            if head % 2 == 0:
                nc.vector.tensor_copy(st_t[:], ps[:])
            else:
                nc.scalar.copy(st_t[:], ps[:])
            nc.sync.dma_start(out_d[head], st_t[:])

        def emit_vc(t):
            h = t // NT
            vc_ps = vcps.tile([P, D], F32, name="vc_ps", tag="vc")
            for st in range(NT):
                nc.tensor.matmul(
                    vc_ps[:],
                    mvt_s[:, st * KVL + t * P: st * KVL + (t + 1) * P],
                    kv3[:, st * 6 * D + (HK + h) * D:
                        st * 6 * D + (HK + h + 1) * D],
                    start=(st == 0), stop=(st == NT - 1))
            nc.vector.tensor_copy(vca[:, t * DP1: t * DP1 + D], vc_ps[:])

        # sweep 0: scores/exp for pair 0 with the vc matmuls interleaved
        # (PE filler under the engine-paced exp window)
        prev = []
        with tc.tile_pool(name="vcps", bufs=2, space="PSUM") as vcps:
            for t in range(NKT):
                prev.append(emit_scores(0, t))
                emit_vc(t)

        # sweeps 1..6: scores/exp for pair p + deferred ctx chains for
        # pair p-1 (A then B) sharing one psum accumulator
        with tc.tile_pool(name="ctxps", bufs=1, space="PSUM") as ctxps:
            for p in range(1, NPAIR + 1):
                cur = []
                ps = None
                for t in range(NKT):
                    if p < NPAIR:
                        cur.append(emit_scores(p, t))
                    if t < 9:   # ctx A: tiles 2t, 2t+1
                        if t == 0:
                            ps = ctxps.tile([DP1, S], F32, name="ctx",
                                            tag="ctx")
                        emit_ctx(2 * t, prev[2 * t], 0, ps,
                                 first=(t == 0), last=False)
                        emit_ctx(2 * t + 1, prev[2 * t + 1], 0, ps,
                                 first=False, last=(t == 8))
                        if t == 8:
                            flush_ctx(ps, 2 * (p - 1))
                    else:       # ctx B: tiles 2(t-9), 2(t-9)+1
                        tb = t - 9
                        if tb == 0:
                            ps = ctxps.tile([DP1, S], F32, name="ctx",
                                            tag="ctx")
                        emit_ctx(2 * tb, prev[2 * tb], 1, ps,
                                 first=(tb == 0), last=False)
                        emit_ctx(2 * tb + 1, prev[2 * tb + 1], 1, ps,
                                 first=False, last=(tb == 8))
                        if tb == 8:
                            flush_ctx(ps, 2 * (p - 1) + 1)
                prev = cur

    nc.compile()
    return nc


def _get_program():
    global _PROGRAM
    if _PROGRAM is None:
        _PROGRAM = _build_program()
    return _PROGRAM


def kernel(hidden_states, attention_mask, Wq, bq, Wk, bk, Wv, bv, gate,
           mem_keys, mem_values):
    from concourse.bass_utils import run_bass_kernel_spmd

    global LAST_RESULTS

    f32, f16 = np.float32, np.float16
    hidden_states = np.asarray(hidden_states, f32)
    attention_mask = np.asarray(attention_mask, f32)
    Wq = np.asarray(Wq, f32)
    bq = np.asarray(bq, f32)
    Wk = np.asarray(Wk, f32)
    bk = np.asarray(bk, f32)
    Wv = np.asarray(Wv, f32)
    bv = np.asarray(bv, f32)
    gate = np.asarray(gate, f32)
    mem_keys = np.asarray(mem_keys, f32)
    mem_values = np.asarray(mem_values, f32)

    hT16 = [np.ascontiguousarray(hidden_states[b].T).astype(f16)
            for b in range(B)]
    wq16 = Wq.astype(f16)
    bq_dev = np.ascontiguousarray(
        bq.reshape(NPAIR, 2, D).transpose(1, 2, 0).reshape(P, NPAIR))
    em_full = np.exp(attention_mask.reshape(B, NH * S)).astype(f32)
    ones_dev = np.ones((1, S), f16)

    in_maps = []
    for c in range(NCORES):
        b, quart = c // NQUART, c % NQUART
        heads = [HK * quart + j for j in range(HK)]
        wkv_c = np.concatenate(
            [Wk[:, h * D:(h + 1) * D] for h in heads]
            + [Wv[:, h * D:(h + 1) * D] for h in heads], axis=1)
        bkv_c = np.concatenate(
            [bk[h * D:(h + 1) * D] for h in heads]
            + [bv[h * D:(h + 1) * D] for h in heads])[None, :]
        # mkt[s, h_local*768+kv] = mem_keys[heads[h_local], kv, s]
        mkt_c = mem_keys[heads].transpose(2, 0, 1).reshape(S, KVL)
        em_c = em_full[b, quart * KVL:(quart + 1) * KVL]
        mvt_c = (mem_values[heads].transpose(2, 0, 1).reshape(S, KVL)
                 * em_c[None, :])
        em_dev = np.ascontiguousarray(em_c.reshape(NKT, P).T)
        in_maps.append({
            "hT": hT16[b],
            "wq": wq16,
            "bq_d": bq_dev,
            "wkv": np.ascontiguousarray(wkv_c).astype(f16),
            "bkv_d": np.ascontiguousarray(bkv_c).astype(f16),
            "mkt": np.ascontiguousarray(mkt_c).astype(f16),
            "mvt": np.ascontiguousarray(mvt_c).astype(f16),
            "em_d": em_dev.astype(f32),
            "em16_d": em_dev.astype(f16),
            "ones_d": ones_dev,
        })

    nc = _get_program()
    res = run_bass_kernel_spmd(nc, in_maps, core_ids=list(range(NCORES)),
                               trace=TRACE)
    LAST_RESULTS = res

    out = np.empty((B, S, NH, D), f32)
    for b in range(B):
        parts = res.results[b * NQUART]["out_d"].astype(f32).copy()
        for c in range(b * NQUART + 1, (b + 1) * NQUART):
            parts += res.results[c]["out_d"]
        num = parts[:, :D, :]                     # [12, 64, 768]
        den = parts[:, D, :]                      # [12, 768]
        ctxT = num / den[:, None, :]
        out[b] = ctxT.transpose(2, 0, 1)          # [768, 12, 64]
    g = (1.0 / (1.0 + np.exp(-gate))).reshape(1, 1, NH, 1)
    return (g * out).astype(f32)
